# revision 1
# baseline (speedup 1.0000x reference)
"""Average Hausdorff loss on 8 Trainium2 NeuronCores.

Strategy
--------
Host (numpy, cheap): binarize masks, 3x3-erosion edge detection, compact
edge-pixel coordinates per (b, c) pair, build "augmented" coordinate
matrices so that a single K=6 bf16 matmul on the PE array produces the
exact value  -(squared distance)/4  for a [128 gth-pts, N pred-pts] tile
in PSUM (all products/partial sums are integers*0.25 < 2^24 -> exact
fp32; coords are centered so byte-split squared norms fit bf16 exactly).

Device (raw Bass, SPMD over 8 cores, 2 (b,c) pairs per core), pipelined
over PE -> ACT -> DVE per [128 gth x 1536 pred] chunk:
  PE : 3 matmuls -> PSUM = -(d^2)/4
  ACT: activation Copy with scale 2^-12 -> SBUF fp16 (sole PSUM reader)
  DVE: two fp16 2x halving folds + short reduce-max -> gth->pred NN,
       one fp16 2x tensor_max accumulate -> pred->gth NN
Host: final partition reduce for the pred->gth direction, sqrt, masked
means, nanmean -- tiny.

Pad points use a far sentinel coordinate so they never win a max.
"""

import numpy as np

H = 256
W = 256
BC = 16          # B*C pairs
N_CORES = 8
PAIRS_PER_CORE = 2
P_CHUNK = 1536   # pred points per DVE op (3 PSUM banks)
G_TILE = 128     # gth points per PE tile (PSUM partitions)
SENT = 16384.0   # sentinel coordinate (centered space), 2^14
D2_SCALE = 2.0 ** -12   # extra scale on -(d^2)/4 so fp16 never overflows
D2_BACK = -4.0 * 4096.0  # value -> d^2


def _edge_maps(x):
    """[BC, H, W] float -> bool edge maps, matching the reference:
    edge = mask & ~erode3x3(mask), erosion padded with True."""
    m = x > 0.5
    p = np.pad(m, ((0, 0), (1, 1), (1, 1)), constant_values=True)
    e = np.ones_like(m)
    for dy in range(3):
        for dx in range(3):
            e &= p[:, dy:dy + H, dx:dx + W]
    return m & ~e


def _compact_coords(edge):
    """bool [H, W] -> (cy, cx) float32 arrays of centered coords."""
    ys, xs = np.nonzero(edge)
    return (ys.astype(np.float32) - 128.0), (xs.astype(np.float32) - 128.0)


def _aug_g(cy, cx, n_pad):
    """lhsT rows [6, n_pad] for the stationary (gth) operand."""
    n = cy.shape[0]
    out = np.zeros((6, n_pad), np.float32)
    fy = np.full(n_pad, SENT, np.float32)
    fx = np.full(n_pad, SENT, np.float32)
    fy[:n] = cy
    fx[:n] = cx
    sq = fy * fy + fx * fx
    b1 = np.floor(sq / 256.0)
    b0 = sq - b1 * 256.0
    out[0] = fy * 0.5
    out[1] = fx * 0.5
    out[2] = -b1
    out[3] = -b0
    out[4] = -64.0
    out[5] = -0.25
    return out


def _aug_p(cy, cx, n_pad):
    """rhs rows [6, n_pad] for the moving (pred) operand."""
    n = cy.shape[0]
    out = np.zeros((6, n_pad), np.float32)
    fy = np.full(n_pad, SENT, np.float32)
    fx = np.full(n_pad, SENT, np.float32)
    fy[:n] = cy
    fx[:n] = cx
    sq = fy * fy + fx * fx
    b1 = np.floor(sq / 256.0)
    b0 = sq - b1 * 256.0
    out[0] = fy
    out[1] = fx
    out[2] = 64.0
    out[3] = 0.25
    out[4] = b1
    out[5] = b0
    return out


def _build_program(structure, self_waits=False):
    """structure: tuple of (n_gtiles, n_pchunks) per pair slot.

    Raw-bass program (no Tile): explicit semaphores, standalone waits.
    This walrus build rejects matmuls carrying >1 inline sync-wait, so
    the streams are arranged such that every instruction needs at most
    one cross-engine wait, emitted as its own EventSemaphore.

    self_waits adds same-engine DVE waits for RAW/WAR chains. Hardware
    orders these via the engine FIFO + per-op pipeline drain; the waits
    exist only to satisfy CoreSim's race detector (sim builds).
    """
    from contextlib import ExitStack
    import concourse.bass as bass
    import concourse.mybir as mybir

    f32 = mybir.dt.float32
    f16 = mybir.dt.float16
    bf16 = mybir.dt.bfloat16
    MAX = mybir.AluOpType.max

    nc = bass.Bass()

    gaug_d, paug_d, dg_d, dp_d = [], [], [], []
    for s, (tg, npc) in enumerate(structure):
        ng_pad = tg * G_TILE
        np_pad = npc * P_CHUNK
        gaug_d.append(nc.declare_dram_parameter(f"gaug{s}", [6, ng_pad], bf16,
                                                isOutput=False))
        paug_d.append(nc.declare_dram_parameter(f"paug{s}", [6, np_pad], bf16,
                                                isOutput=False))
        dg_d.append(nc.declare_dram_parameter(f"dg{s}", [G_TILE, tg], f32,
                                              isOutput=True))
        dp_d.append(nc.declare_dram_parameter(f"dp{s}", [G_TILE, np_pad], f16,
                                              isOutput=True))

    n_slots = len(structure)
    total_chunks = sum(tg * npc for tg, npc in structure)
    NB = 4  # d2s fp16 ring depth

    with ExitStack() as ctx:
        gs, ps, dp_acc, dg_st, dg_all = [], [], [], [], []
        for s, (tg, npc) in enumerate(structure):
            gs.append(ctx.enter_context(
                nc.sbuf_tensor(f"gs{s}", [6, tg * G_TILE], bf16)))
            ps.append(ctx.enter_context(
                nc.sbuf_tensor(f"ps{s}", [6, npc * P_CHUNK], bf16)))
            dp_acc.append(ctx.enter_context(
                nc.sbuf_tensor(f"dpacc{s}", [G_TILE, npc * P_CHUNK], f16)))
            dg_st.append(ctx.enter_context(
                nc.sbuf_tensor(f"dgst{s}", [G_TILE, tg, npc], f32)))
            dg_all.append(ctx.enter_context(
                nc.sbuf_tensor(f"dgall{s}", [G_TILE, tg], f32)))
        pt = [ctx.enter_context(nc.psum_tensor(f"pt{i}", [G_TILE, P_CHUNK], f32))
              for i in range(2)]
        # fp16 distance ring: 4 chunk slots in one tensor so adjacent pairs
        # (even k, odd k) can be consumed by single wide DVE ops.
        d2s = ctx.enter_context(
            nc.sbuf_tensor("d2s", [G_TILE, NB, P_CHUNK], f16))
        # fold buffers for the dg reduction (fp16 tt_max halving steps)
        fd1 = [ctx.enter_context(
            nc.sbuf_tensor(f"fd1_{i}", [G_TILE, 2, P_CHUNK // 2], f16))
            for i in range(2)]
        fd2 = [ctx.enter_context(
            nc.sbuf_tensor(f"fd2_{i}", [G_TILE, 2, P_CHUNK // 4], f16))
            for i in range(2)]
        fd3 = [ctx.enter_context(
            nc.sbuf_tensor(f"fd3_{i}", [G_TILE, P_CHUNK // 4], f16))
            for i in range(2)]
        fd4 = [ctx.enter_context(
            nc.sbuf_tensor(f"fd4_{i}", [G_TILE, P_CHUNK // 8], f16))
            for i in range(2)]

        dma_sems = [ctx.enter_context(nc.semaphore(f"dma_in{s}"))
                    for s in range(n_slots)]
        pe_sem = ctx.enter_context(nc.semaphore("pe_done"))
        act_sem = ctx.enter_context(nc.semaphore("act_done"))
        dve_sem = ctx.enter_context(nc.semaphore("dve_done"))
        out_sem = ctx.enter_context(nc.semaphore("dma_out"))
        block = ctx.enter_context(nc.Block())

        # Dry run of the DVE emission to get exact dve_sem values.
        # Groups: one per (slot, gt). npc==2 groups use paired (3072-wide)
        # DVE ops; other npc use per-chunk ops. 4 DVE incs per chunk-pair /
        # per chunk respectively; +1 final dg reduce per slot.
        chunk_last_read = []   # per chunk k: dve_sem when its d2s reads done
        slot_end = []
        _n = 0
        _k = 0
        for tg, npc in structure:
            paired = (npc == 2 and _k % 2 == 0)
            for gt in range(tg):
                if paired:
                    # flat group: 4 folds + reduce + dp max = 6 ops
                    _n += 6
                    chunk_last_read += [_n, _n]
                    _k += 2
                else:
                    for _ in range(npc):
                        _n += 4
                        chunk_last_read.append(_n)
                        _k += 1
            if not paired:
                _n += 1  # slot-final dg reduce (fallback path only)
            slot_end.append(_n)

        @block.sync
        def _(sync):
            for s in range(n_slots):
                sync.dma_start(gs[s][:], gaug_d[s][:]).then_inc(dma_sems[s], 16)
                sync.dma_start(ps[s][:], paug_d[s][:]).then_inc(dma_sems[s], 16)
            for s in range(n_slots):
                sync.wait_ge(dve_sem, slot_end[s])
                sync.dma_start(dg_d[s][:], dg_all[s][:]).then_inc(out_sem, 16)
                sync.dma_start(dp_d[s][:], dp_acc[s][:]).then_inc(out_sem, 16)
            # No final out_sem wait: the block-end drain waits the DMA
            # HW queues, so output completion is already guaranteed.

        @block.tensor
        def _(tensor):
            k = 0
            for s, (tg, npc) in enumerate(structure):
                # start as soon as THIS slot's inputs have landed
                tensor.wait_ge(dma_sems[s], 32)
                for gt in range(tg):
                    lhsT = gs[s][:, gt * G_TILE:(gt + 1) * G_TILE]
                    for pc in range(npc):
                        if k >= 2:
                            # psum slot reuse: ACT (sole PSUM reader) of
                            # chunk k-2 done
                            tensor.wait_ge(act_sem, k - 1)
                        p = pt[k % 2]
                        for b in range(P_CHUNK // 512):
                            off = pc * P_CHUNK + b * 512
                            mm = nc.tensor.matmul(
                                p[:, b * 512:(b + 1) * 512],
                                lhsT,
                                ps[s][:, off:off + 512],
                                start=True, stop=True,
                            )
                        mm.then_inc(pe_sem, 1)
                        k += 1

        @block.scalar
        def _(scalar):
            # PSUM fp32 -> SBUF fp16, scaled by 2^-12 so sentinel-pad
            # distances stay finite in fp16 (power-of-2: real values
            # keep their mantissa exactly).
            for k in range(total_chunks):
                scalar.wait_ge(pe_sem, k + 1)
                if k >= NB:
                    scalar.wait_ge(dve_sem, chunk_last_read[k - NB])
                nc.scalar.activation(
                    d2s[:, k % NB, :], pt[k % 2][:],
                    mybir.ActivationFunctionType.Copy, scale=D2_SCALE,
                ).then_inc(act_sem, 1)

        @block.vector
        def _(vector):
            H1 = P_CHUNK // 2
            H2 = P_CHUNK // 4
            k = 0
            n_ops = 0
            gi = 0            # group (gt) counter, for fold ring indexing
            writer = {}       # dp_acc region -> op count of its last write
            f_free = {}       # fold ring slot -> op count after its last read

            def dg_fold(din0, din1, f1, f1a, f1b, f2, out_col, ring):
                """fold-fold-reduce: d halves -> f1 -> f2 -> reduce."""
                nonlocal n_ops
                w = f_free.get(("f1", ring))
                if self_waits and w:
                    vector.wait_ge(dve_sem, w)  # f1 ring WAR
                nc.vector.tensor_max(f1, din0, din1).then_inc(dve_sem, 1)
                n_ops += 1
                w = f_free.get(("f2", ring))
                if self_waits:
                    vector.wait_ge(dve_sem, max(n_ops, w or 0))
                nc.vector.tensor_max(f2, f1a, f1b).then_inc(dve_sem, 1)
                n_ops += 1
                f_free[("f1", ring)] = n_ops
                if self_waits:
                    vector.wait_ge(dve_sem, n_ops)  # f2 RAW
                nc.vector.tensor_reduce(
                    out_col, f2, axis=mybir.AxisListType.X, op=MAX,
                ).then_inc(dve_sem, 1)
                n_ops += 1
                f_free[("f2", ring)] = n_ops

            def dp_accum(dpc, src, first):
                nonlocal n_ops
                if first:
                    ins = nc.vector.tensor_copy(dpc, src)
                else:
                    if self_waits:
                        vector.wait_ge(dve_sem, writer[id(dpc.tensor)])
                    ins = nc.vector.tensor_max(dpc, dpc, src)
                ins.then_inc(dve_sem, 1)
                n_ops += 1

            for s, (tg, npc) in enumerate(structure):
                paired = (npc == 2 and k % 2 == 0)
                for gt in range(tg):
                    r = gi % 2
                    if paired:
                        pr = k % NB  # even, pair occupies slots pr, pr+1
                        vector.wait_ge(act_sem, k + 2)
                        dpair = d2s[:, pr:pr + 2, :].rearrange("p a b -> p (a b)")
                        # flat fold chain over the whole 3072-wide group:
                        # each step halves at fp16 2x; tiny 1x reduce last.
                        chain = [
                            fd1[r][:].rearrange("p a b -> p (a b)"),
                            fd2[r][:].rearrange("p a b -> p (a b)"),
                            fd3[r][:],
                            fd4[r][:],
                        ]
                        src = dpair
                        W = 2 * P_CHUNK
                        for buf in chain:
                            if self_waits:
                                vector.wait_ge(dve_sem, n_ops)
                            nc.vector.tensor_max(
                                buf[:, 0:W // 2],
                                src[:, 0:W // 2], src[:, W // 2:W],
                            ).then_inc(dve_sem, 1)
                            n_ops += 1
                            src = buf
                            W //= 2
                        if self_waits:
                            vector.wait_ge(dve_sem, n_ops)
                        nc.vector.tensor_reduce(
                            dg_all[s][:, gt:gt + 1], src[:, 0:W],
                            axis=mybir.AxisListType.X, op=MAX,
                        ).then_inc(dve_sem, 1)
                        n_ops += 1
                        dpc = dp_acc[s][:, 0:2 * P_CHUNK]
                        dp_accum(dpc, dpair, gt == 0)
                        writer[id(dpc.tensor)] = n_ops
                        k += 2
                    else:
                        for pc in range(npc):
                            vector.wait_ge(act_sem, k + 1)
                            c = k % NB
                            f1 = fd1[r][:, 0, :]
                            f2 = fd2[r][:, 0, :]
                            dg_fold(
                                d2s[:, c, 0:H1], d2s[:, c, H1:P_CHUNK],
                                f1, f1[:, 0:H2], f1[:, H2:H1],
                                f2, dg_st[s][:, gt, pc:pc + 1], r,
                            )
                            dpc = dp_acc[s][:, pc * P_CHUNK:(pc + 1) * P_CHUNK]
                            dp_accum(dpc, d2s[:, c, :], gt == 0)
                            writer[id(dpc.tensor)] = n_ops
                            k += 1
                    gi += 1
                if not paired:
                    if self_waits:
                        vector.wait_ge(dve_sem, n_ops)  # dg_st writes done
                    nc.vector.tensor_reduce(
                        dg_all[s][:], dg_st[s][:],
                        axis=mybir.AxisListType.X, op=MAX,
                    ).then_inc(dve_sem, 1)
                    n_ops += 1

    return nc


def _loss_from_nn(dg_val, dp_val, n_g, n_p):
    """Mirror the reference combination. dg_val/dp_val are the device maxes
    of -(d^2)/4 * 2^-12 for the first n_g / n_p (valid) points."""
    with np.errstate(divide="ignore", invalid="ignore", over="ignore"):
        d_g = np.sqrt(np.maximum(D2_BACK * dg_val.astype(np.float64), 0.0))
        d_p = np.sqrt(np.maximum(D2_BACK * dp_val.astype(np.float64), 0.0))
        gth2pred = d_g.sum() / n_g if n_g > 0 else np.float64(np.nan)
        pred2gth = d_p.sum() / n_p if n_p > 0 else np.float64(np.nan)
        ahd = (gth2pred + pred2gth) / 2.0
        if n_g == 0 and n_p == 0:
            ahd = np.float64(np.nan)
        return 1.0 - 1.0 / (1.0 + ahd)


RUN_OPTS = {}    # extra kwargs for run_bass_kernel_spmd (test harness hook)
LAST_RES = None  # last BassKernelResults (test harness hook)


def kernel(gth, pred):
    from concourse.bass_utils import run_bass_kernel_spmd
    import ml_dtypes

    gth = np.asarray(gth, np.float32).reshape(BC, H, W)
    pred = np.asarray(pred, np.float32).reshape(BC, H, W)

    gedge = _edge_maps(gth)
    pedge = _edge_maps(pred)
    pts = []
    for i in range(BC):
        gy, gx = _compact_coords(gedge[i])
        py, px = _compact_coords(pedge[i])
        pts.append((gy, gx, py, px))

    # Balance pairs across cores: sort by tile cost, big+small per core.
    def cost(i):
        gy = pts[i][0]
        py = pts[i][2]
        return (max(1, -(-len(gy) // G_TILE)) * max(1, -(-len(py) // P_CHUNK)))
    order = sorted(range(BC), key=cost, reverse=True)
    assign = [[order[c], order[BC - 1 - c]] for c in range(N_CORES)]

    # Uniform per-slot structure = max over cores.
    structure = []
    for s in range(PAIRS_PER_CORE):
        tg = max(max(1, -(-len(pts[assign[c][s]][0]) // G_TILE))
                 for c in range(N_CORES))
        npc = max(max(1, -(-len(pts[assign[c][s]][2]) // P_CHUNK))
                  for c in range(N_CORES))
        structure.append((tg, npc))
    structure = tuple(structure)

    nc = _build_program(structure)

    in_maps = []
    for c in range(N_CORES):
        m = {}
        for s in range(PAIRS_PER_CORE):
            tg, npc = structure[s]
            gy, gx, py, px = pts[assign[c][s]]
            m[f"gaug{s}"] = _aug_g(gy, gx, tg * G_TILE).astype(ml_dtypes.bfloat16)
            m[f"paug{s}"] = _aug_p(py, px, npc * P_CHUNK).astype(ml_dtypes.bfloat16)
        in_maps.append(m)

    res = run_bass_kernel_spmd(nc, in_maps, list(range(N_CORES)), **RUN_OPTS)
    global LAST_RES
    LAST_RES = res
    results = res.results

    losses = np.full(BC, np.nan, np.float64)
    for c in range(N_CORES):
        for s in range(PAIRS_PER_CORE):
            i = assign[c][s]
            gy, gx, py, px = pts[i]
            n_g, n_p = len(gy), len(py)
            dg = np.asarray(results[c][f"dg{s}"], np.float64)   # [128, tg]
            dp = np.asarray(results[c][f"dp{s}"], np.float64)   # [128, np_pad]
            dg_flat = dg.T.reshape(-1)[:n_g]
            dp_red = dp.max(axis=0)[:n_p]
            losses[i] = _loss_from_nn(dg_flat, dp_red, n_g, n_p)

    return np.float32(np.nanmean(losses.astype(np.float32)))



# revision 5
# speedup vs baseline: 2.6744x; 2.6744x over previous
"""Average Hausdorff loss on 8 Trainium2 NeuronCores — banded KNN version.

Strategy
--------
Host (numpy, cheap): binarize masks, 3x3-erosion edge detection, compact
edge-pixel coordinates per (b, c) pair.  A half-resolution two-pass EDT
gives a certified upper bound u(g) on every point's NN distance; from
those bounds each 128-point gth tile gets a *band* — a contiguous pred
index interval guaranteed to contain every tile point's true NN (and,
symmetrically, to cover every pred point whose NN could be in the tile).
Mean band ~600-1000 columns instead of the full 3072: the device only
computes the banded distance matrix.

Device (raw Bass, SPMD over 8 cores, 2 (b,c) pairs per core):
  PE : per tile, matmuls of the 6-row augmented operands over the band
       -> PSUM = -(d^2)/4 (exact in fp32, see _aug_g/_aug_p)
  ACT: activation Copy with scale 2^-12 -> SBUF fp16, one op per 2-tile
       PSUM group
  DVE: batched 4-tile fp16 halving folds (gth->pred NN partials) and
       per-tile tensor_max accumulate into a global pred-space dp map
Host: final small reductions (W/4-wide row max per tile, 128-part col
max for dp), sqrt, masked means, nanmean.

Pad gth rows / pred cols use a far sentinel coordinate so they never
win a max; bands are clamped so matmuls never read out of range.
"""

import numpy as np

H = 256
W_IMG = 256
BC = 16          # B*C pairs
N_CORES = 8
SLOTS = 2        # pairs per core
G_TILE = 128     # gth points per PE tile (PSUM partitions)
NP_PAD = 3072    # padded pred points (global space)
QUANT = 128      # band quantization
W_CAP = 1024     # max band width per tile-slot (PSUM: 2 tiles <= 2048 fp32)
FOLD_B = 4       # tiles per DVE fold group
SENT = 16384.0   # sentinel coordinate (centered space), 2^14
D2_SCALE = 2.0 ** -12   # scale on -(d^2)/4 so fp16 never overflows
D2_BACK = -4.0 * 4096.0  # value -> d^2
DP_INIT = -60000.0       # dp_acc init; loses to every real/sentinel value
EDT_SLACK = 4.5  # certified slack for half-res EDT upper bound


def _edge_maps(x):
    """[BC, H, W] float -> bool edge maps, matching the reference."""
    m = x > 0.5
    p = np.pad(m, ((0, 0), (1, 1), (1, 1)), constant_values=True)
    e = np.ones_like(m)
    for dy in range(3):
        for dx in range(3):
            e &= p[:, dy:dy + H, dx:dx + W_IMG]
    return m & ~e


def _edt_half(mask):
    """Exact EDT of the half-res occupancy of `mask` ([256,256] bool).
    Returns [128,128] float32 of half-res euclidean distances."""
    m = mask.reshape(128, 2, 128, 2).any(axis=(1, 3))
    BIG = np.float32(1e9)
    col = np.where(m, np.float32(0.0), BIG)           # [y, x]
    ar = np.arange(128, dtype=np.float32)
    d2 = (ar[:, None] - ar[None, :]) ** 2             # [out, in]
    D1 = (d2[:, :, None] + col[None, :, :]).min(1)    # [y, x]
    D2 = (D1[:, None, :] + d2[None, :, :]).min(2)     # [y, x]
    return np.sqrt(D2)


def _nn_upper_bound(edt_half_other, ys, xs):
    """Certified upper bound on distance to the other point set."""
    return 2.0 * edt_half_other[ys // 2, xs // 2] + EDT_SLACK


def _aug_g(cy, cx, n_pad):
    """lhsT rows [6, n_pad] for the stationary (gth) operand.
    cy/cx may contain sentinel entries already (pad slots)."""
    out = np.zeros((6, n_pad), np.float32)
    fy, fx = cy, cx
    sq = fy * fy + fx * fx
    b1 = np.floor(sq / 256.0)
    b0 = sq - b1 * 256.0
    out[0] = fy * 0.5
    out[1] = fx * 0.5
    out[2] = -b1
    out[3] = -b0
    out[4] = -64.0
    out[5] = -0.25
    return out


def _aug_p(cy, cx, n_pad):
    """rhs rows [6, n_pad] for the moving (pred) operand."""
    n = cy.shape[0]
    out = np.zeros((6, n_pad), np.float32)
    fy = np.full(n_pad, SENT, np.float32)
    fx = np.full(n_pad, SENT, np.float32)
    fy[:n] = cy
    fx[:n] = cx
    sq = fy * fy + fx * fx
    b1 = np.floor(sq / 256.0)
    b0 = sq - b1 * 256.0
    out[0] = fy
    out[1] = fx
    out[2] = 64.0
    out[3] = 0.25
    out[4] = b1
    out[5] = b0
    return out


def _pair_bands(gy, gx, py, px, u_g, v_p, T):
    """Per-tile pred index requirement intervals for one pair.

    Tiles are gth quantiles: tile t covers sorted gth indices
    [t*n_g//T, (t+1)*n_g//T).  Returns list of (lo, hi) pred-index
    intervals covering both directions' NN requirements."""
    n_g, n_p = len(gy), len(py)
    bands = []
    for t in range(T):
        a, b = (t * n_g) // T, ((t + 1) * n_g) // T
        if b <= a:
            bands.append((0, 0))
            continue
        ymin, ymax = gy[a:b].min(), gy[a:b].max()
        U = u_g[a:b].max()
        # dg: all pred with y in [ymin-U, ymax+U]
        lo1 = np.searchsorted(py, ymin - U, 'left')
        hi1 = np.searchsorted(py, ymax + U, 'right')
        # dp coverage: pred p whose v_p-ball in y intersects [ymin, ymax]
        sel = (py + v_p >= ymin) & (py - v_p <= ymax)
        nz = np.nonzero(sel)[0]
        if len(nz):
            lo2, hi2 = nz[0], nz[-1] + 1
        else:
            lo2, hi2 = lo1, hi1
        lo, hi = min(lo1, lo2), max(hi1, hi2)
        bands.append((int(lo), int(hi)))
    return bands


def _build_schedule(all_bands, T):
    """all_bands: [SLOTS][8][T] of (lo, hi) in real pred index space.

    Returns per slot a list of emission jobs:
      (tile_t, Q, W, split_idx)  with Q, W quantized, W <= W_CAP,
    sorted by W descending, plus fold-group padding (W equalized in
    groups of FOLD_B)."""
    sched = []
    for s in range(SLOTS):
        jobs = []
        for t in range(T):
            lo = min(b[t][0] for b in all_bands[s])
            hi = max(b[t][1] for b in all_bands[s])
            if hi <= lo:
                hi = lo + 1
            q = (lo // QUANT) * QUANT
            qh = -(-hi // QUANT) * QUANT
            qh = min(qh, NP_PAD)
            q = min(q, qh - QUANT)
            wid = qh - q
            # split wide tiles into <= W_CAP windows
            n_sp = -(-wid // W_CAP)
            base = wid // n_sp
            base = -(-base // QUANT) * QUANT
            off = q
            for i in range(n_sp):
                w = min(base, qh - off)
                if w <= 0:
                    break
                jobs.append([t, off, w, i])
                off += w
        # sort by width desc for fold-group uniformity
        jobs.sort(key=lambda j: -j[2])
        # pad each fold group to its max width (= first job's width),
        # re-clamping Q so Q+W stays in range
        for g0 in range(0, len(jobs), FOLD_B):
            grp = jobs[g0:g0 + FOLD_B]
            wmax = grp[0][2]
            for j in grp:
                if j[2] < wmax:
                    j[1] = max(0, min(j[1], NP_PAD - wmax))
                    j[2] = wmax
        sched.append(jobs)
    return sched


def _build_program(slot_jobs):
    """slot_jobs: per slot, list of (tile, Q, W, split) emission jobs.
    Raw-bass program: explicit semaphores, standalone waits."""
    from contextlib import ExitStack
    import concourse.bass as bass
    import concourse.mybir as mybir

    f32 = mybir.dt.float32
    f16 = mybir.dt.float16
    bf16 = mybir.dt.bfloat16
    MAX = mybir.AluOpType.max

    nc = bass.Bass()

    T_rows = [max(j[0] for j in jobs) + 1 for jobs in slot_jobs]
    # dg_st packing offsets (per slot, per job): W/4 columns each
    dg_ofs, dg_tot = [], []
    for s, jobs in enumerate(slot_jobs):
        ofs, tot = [], 0
        for j in jobs:
            ofs.append(tot)
            tot += j[2] // 4
        dg_ofs.append(ofs)
        dg_tot.append(tot)
    wmax = max(max(j[2] for j in jobs) for jobs in slot_jobs)

    gaug_d, paug_d, dg_d, dp_d = [], [], [], []
    for s, jobs in enumerate(slot_jobs):
        gaug_d.append(nc.declare_dram_parameter(
            f"gaug{s}", [6, T_rows[s] * G_TILE], bf16, isOutput=False))
        paug_d.append(nc.declare_dram_parameter(
            f"paug{s}", [6, NP_PAD], bf16, isOutput=False))
        dg_d.append(nc.declare_dram_parameter(
            f"dg{s}", [G_TILE, dg_tot[s]], f16, isOutput=True))
        dp_d.append(nc.declare_dram_parameter(
            f"dp{s}", [G_TILE, NP_PAD], f16, isOutput=True))

    with ExitStack() as ctx:
        gs, ps, dp_acc, dg_st = [], [], [], []
        for s in range(SLOTS):
            gs.append(ctx.enter_context(
                nc.sbuf_tensor(f"gs{s}", [6, T_rows[s] * G_TILE], bf16)))
            ps.append(ctx.enter_context(
                nc.sbuf_tensor(f"ps{s}", [6, NP_PAD], bf16)))
            dp_acc.append(ctx.enter_context(
                nc.sbuf_tensor(f"dpacc{s}", [G_TILE, NP_PAD], f16)))
            dg_st.append(ctx.enter_context(
                nc.sbuf_tensor(f"dgst{s}", [G_TILE, dg_tot[s]], f16)))
        # 2 PSUM tensors, [128, 2048] each (4 banks) -> one per 2-tile group
        pt = [ctx.enter_context(nc.psum_tensor(f"pt{i}", [G_TILE, 2048], f32))
              for i in range(2)]
        # fp16 block ring: 2 fold-group slots x FOLD_B tiles x wmax
        d2s = ctx.enter_context(
            nc.sbuf_tensor("d2s", [G_TILE, 2, FOLD_B, wmax], f16))
        fd1 = ctx.enter_context(
            nc.sbuf_tensor("fd1", [G_TILE, 2, FOLD_B, wmax // 2], f16))

        dma_sems = [ctx.enter_context(nc.semaphore(f"dma_in{s}"))
                    for s in range(SLOTS)]
        pe_sem = ctx.enter_context(nc.semaphore("pe_done"))
        act_sem = ctx.enter_context(nc.semaphore("act_done"))
        dve_sem = ctx.enter_context(nc.semaphore("dve_done"))
        out_sem = ctx.enter_context(nc.semaphore("dma_out"))
        block = ctx.enter_context(nc.Block())

        # ---- emission-order bookkeeping (shared by all engine closures)
        # psum groups: pairs of consecutive jobs in emission order (per slot,
        # globally numbered);  fold groups: pairs of psum groups.
        pg_list = []   # (slot, [job indices])
        for s, jobs in enumerate(slot_jobs):
            for i in range(0, len(jobs), 2):
                pg_list.append((s, list(range(i, min(i + 2, len(jobs))))))
        fg_list = []   # (slot, [job indices], [pg ids])
        pgi = 0
        for s, jobs in enumerate(slot_jobs):
            i = 0
            while i < len(jobs):
                take = min(FOLD_B, len(jobs) - i)
                pgs = [pgi, pgi + 1] if take > 2 else [pgi]
                fg_list.append((s, list(range(i, i + take)), pgs))
                pgi += len(pgs)
                i += take

        # DVE op counting for ring/psum reuse and output DMA:
        # per fold group: 1 memset? no -- memsets first (2 ops), then per
        # fold group: f1, f2, then len(tiles) dp ops.
        n_dve = [0]
        fg_end = []          # dve_sem value when fold group fully consumed
        slot_dve_end = [0] * SLOTS
        n_dve[0] += SLOTS    # two leading memsets
        for (s, tix, pgs) in fg_list:
            n_dve[0] += 2 + len(tix)
            fg_end.append(n_dve[0])
            slot_dve_end[s] = n_dve[0]

        # act ops: one per psum group
        # pe: one sem inc per psum group (after last matmul of the group)

        @block.sync
        def _(sync):
            for s in range(SLOTS):
                sync.dma_start(gs[s][:], gaug_d[s][:]).then_inc(dma_sems[s], 16)
                sync.dma_start(ps[s][:], paug_d[s][:]).then_inc(dma_sems[s], 16)
            for s in range(SLOTS):
                sync.wait_ge(dve_sem, slot_dve_end[s])
                sync.dma_start(dg_d[s][:], dg_st[s][:]).then_inc(out_sem, 16)
                sync.dma_start(dp_d[s][:], dp_acc[s][:]).then_inc(out_sem, 16)

        @block.tensor
        def _(tensor):
            cur_slot = -1
            for pg, (s, jix) in enumerate(pg_list):
                if s != cur_slot:
                    tensor.wait_ge(dma_sems[s], 32)
                    cur_slot = s
                if pg >= 2:
                    # psum tensor reuse: ACT of group pg-2 done
                    tensor.wait_ge(act_sem, pg - 1)
                jobs = slot_jobs[s]
                mm = None
                for slot_pos, ji in enumerate(jix):
                    t, Q, Wb, _sp = jobs[ji]
                    lhsT = gs[s][:, t * G_TILE:(t + 1) * G_TILE]
                    o = slot_pos * Wb     # psum column offset
                    done = 0
                    while done < Wb:
                        # pieces must not cross PSUM 512-col banks
                        room = 512 - ((o + done) % 512)
                        w = min(room, 512, Wb - done)
                        mm = nc.tensor.matmul(
                            pt[pg % 2][:, o + done:o + done + w],
                            lhsT,
                            ps[s][:, Q + done:Q + done + w],
                            start=True, stop=True,
                        )
                        done += w
                mm.then_inc(pe_sem, 1)

        @block.scalar
        def _(scalar):
            fg_of_pg = {}
            for fgi, (s, tix, pgs) in enumerate(fg_list):
                for p in pgs:
                    fg_of_pg[p] = (fgi, pgs)
            for pg, (s, jix) in enumerate(pg_list):
                scalar.wait_ge(pe_sem, pg + 1)
                fgi, pgs = fg_of_pg[pg]
                if fgi >= 2:
                    # ring slot reuse: fold group fgi-2 fully consumed
                    scalar.wait_ge(dve_sem, fg_end[fgi - 2])
                W_g = slot_jobs[s][jix[0]][2]
                half = pgs.index(pg)   # 0 or 1 within the fold group
                nw = len(jix) * W_g
                dst = d2s[:, fgi % 2, half * 2:half * 2 + len(jix), :W_g]
                src = pt[pg % 2][:, 0:nw].rearrange(
                    "p (a b) -> p a b", a=len(jix))
                nc.scalar.activation(
                    dst, src,
                    mybir.ActivationFunctionType.Copy, scale=D2_SCALE,
                ).then_inc(act_sem, 1)

        @block.vector
        def _(vector):
            for s in range(SLOTS):
                nc.vector.memset(dp_acc[s][:], DP_INIT).then_inc(dve_sem, 1)
            act_count = 0
            for fgi, (s, tix, pgs) in enumerate(fg_list):
                act_count += len(pgs)
                vector.wait_ge(act_sem, act_count)
                jobs = slot_jobs[s]
                W_g = jobs[tix[0]][2]
                nt = len(tix)
                r = fgi % 2
                h1, h2 = W_g // 2, W_g // 4
                # f1: [128, nt, W/2] = max(left half, right half)
                nc.vector.tensor_max(
                    fd1[:, r, 0:nt, 0:h1],
                    d2s[:, r, 0:nt, 0:h1],
                    d2s[:, r, 0:nt, h1:W_g],
                ).then_inc(dve_sem, 1)
                # f2 -> dg_st packed region
                o0 = dg_ofs[s][tix[0]]
                dst = dg_st[s][:, o0:o0 + nt * h2].rearrange(
                    "p (a b) -> p a b", a=nt)
                nc.vector.tensor_max(
                    dst,
                    fd1[:, r, 0:nt, 0:h2],
                    fd1[:, r, 0:nt, h2:h1],
                ).then_inc(dve_sem, 1)
                # dp accumulate per tile
                for slot_pos, ji in enumerate(tix):
                    t, Q, Wb, _sp = jobs[ji]
                    nc.vector.tensor_max(
                        dp_acc[s][:, Q:Q + Wb],
                        dp_acc[s][:, Q:Q + Wb],
                        d2s[:, r, slot_pos, 0:Wb],
                    ).then_inc(dve_sem, 1)

    return nc


def _loss_from_nn(d_g, d_p, n_g, n_p):
    with np.errstate(divide="ignore", invalid="ignore", over="ignore"):
        gth2pred = d_g.sum() / n_g if n_g > 0 else np.float64(np.nan)
        pred2gth = d_p.sum() / n_p if n_p > 0 else np.float64(np.nan)
        ahd = (gth2pred + pred2gth) / 2.0
        if n_g == 0 and n_p == 0:
            ahd = np.float64(np.nan)
        return 1.0 - 1.0 / (1.0 + ahd)


RUN_OPTS = {}    # extra kwargs for run_bass_kernel_spmd (test harness hook)
LAST_RES = None  # last BassKernelResults (test harness hook)
LAST_INFO = {}   # debug info (test harness hook)


def kernel(gth, pred):
    from concourse.bass_utils import run_bass_kernel_spmd
    import ml_dtypes

    gth = np.asarray(gth, np.float32).reshape(BC, H, W_IMG)
    pred = np.asarray(pred, np.float32).reshape(BC, H, W_IMG)

    gedge = _edge_maps(gth)
    pedge = _edge_maps(pred)

    pts, bounds = [], []
    for i in range(BC):
        gy, gx = np.nonzero(gedge[i])
        py, px = np.nonzero(pedge[i])
        pts.append((gy.astype(np.int64), gx.astype(np.int64),
                    py.astype(np.int64), px.astype(np.int64)))
        if len(gy) and len(py):
            ep = _edt_half(pedge[i])
            eg = _edt_half(gedge[i])
            u_g = _nn_upper_bound(ep, gy, gx)   # gth -> pred bound
            v_p = _nn_upper_bound(eg, py, px)   # pred -> gth bound
        else:
            u_g = np.zeros(len(gy)); v_p = np.zeros(len(py))
        bounds.append((u_g, v_p))

    n_gs = [len(p[0]) for p in pts]
    T = max(1, -(-max(n_gs) // G_TILE))

    # Edge-case fallback: if any pair is empty on either side, fall back to
    # full bands (device result rows/cols for it are ignored or trivially
    # handled host-side; with the fixed harness inputs this never triggers).
    full_fallback = [i for i in range(BC) if n_gs[i] == 0 or len(pts[i][2]) == 0]

    # assign pairs to (core, slot): sort by total band cost, deal alternately
    per_pair_bands = []
    for i in range(BC):
        gy, gx, py, px = pts[i]
        u_g, v_p = bounds[i]
        if i in full_fallback:
            per_pair_bands.append([(0, NP_PAD)] * T)
        else:
            per_pair_bands.append(_pair_bands(gy, gx, py, px, u_g, v_p, T))
    cost = [sum(hi - lo for lo, hi in b) for b in per_pair_bands]
    order = sorted(range(BC), key=lambda i: -cost[i])
    slot_pairs = [order[0::2], order[1::2]]          # 8 pairs per slot
    # core c gets slot0[c], slot1[7-c] (anti-correlated sizes)
    assign = [[slot_pairs[0][c], slot_pairs[1][N_CORES - 1 - c]]
              for c in range(N_CORES)]

    all_bands = [[per_pair_bands[i] for i in slot_pairs[0]],
                 [per_pair_bands[i] for i in slot_pairs[1]]]
    slot_jobs = _build_schedule(all_bands, T)

    nc = _build_program(slot_jobs)

    # per-core input data
    in_maps = []
    for c in range(N_CORES):
        m = {}
        for s in range(SLOTS):
            i = assign[c][s]
            gy, gx, py, px = pts[i]
            n_g = len(gy)
            # quantile-tiled gaug with sentinel padding
            cy = np.full(T * G_TILE, SENT, np.float32)
            cx = np.full(T * G_TILE, SENT, np.float32)
            for t in range(T):
                a, b = (t * n_g) // T, ((t + 1) * n_g) // T
                cy[t * G_TILE:t * G_TILE + (b - a)] = gy[a:b] - 128.0
                cx[t * G_TILE:t * G_TILE + (b - a)] = gx[a:b] - 128.0
            m[f"gaug{s}"] = _aug_g(cy, cx, T * G_TILE).astype(ml_dtypes.bfloat16)
            m[f"paug{s}"] = _aug_p(py.astype(np.float32) - 128.0,
                                   px.astype(np.float32) - 128.0,
                                   NP_PAD).astype(ml_dtypes.bfloat16)
        in_maps.append(m)

    res = run_bass_kernel_spmd(nc, in_maps, list(range(N_CORES)), **RUN_OPTS)
    global LAST_RES, LAST_INFO
    LAST_RES = res
    LAST_INFO = {"slot_jobs": slot_jobs, "assign": assign, "T": T}
    results = res.results

    # dg_st offsets (mirror _build_program)
    dg_ofs = []
    for s in range(SLOTS):
        ofs, tot = [], 0
        for j in slot_jobs[s]:
            ofs.append(tot)
            tot += j[2] // 4
        dg_ofs.append(ofs)

    losses = np.full(BC, np.nan, np.float64)
    for c in range(N_CORES):
        for s in range(SLOTS):
            i = assign[c][s]
            gy, gx, py, px = pts[i]
            n_g, n_p = len(gy), len(py)
            if n_g == 0 and n_p == 0:
                continue
            dg_raw = np.asarray(results[c][f"dg{s}"], np.float32)  # [128, tot]
            dp_raw = np.asarray(results[c][f"dp{s}"], np.float32)  # [128, NP_PAD]
            # dg: per (tile,row) max over all split jobs of that tile
            val_g = np.full((T, G_TILE), -np.inf, np.float32)
            for ji, (t, Q, Wb, _sp) in enumerate(slot_jobs[s]):
                o = dg_ofs[s][ji]
                blk = dg_raw[:, o:o + Wb // 4].max(axis=1)   # [128]
                val_g[t] = np.maximum(val_g[t], blk)
            # map tile rows back to real gth points
            dgv = np.empty(n_g, np.float32)
            for t in range(T):
                a, b = (t * n_g) // T, ((t + 1) * n_g) // T
                dgv[a:b] = val_g[t, :b - a]
            dpv = dp_raw.max(axis=0)[:n_p]
            d_g = np.sqrt(np.maximum(D2_BACK * dgv.astype(np.float64), 0.0))
            d_p = np.sqrt(np.maximum(D2_BACK * dpv.astype(np.float64), 0.0))
            losses[i] = _loss_from_nn(d_g, d_p, n_g, n_p)

    return np.float32(np.nanmean(losses.astype(np.float32)))


# revision 8
# speedup vs baseline: 3.3435x; 1.2502x over previous
"""Average Hausdorff loss on 8 Trainium2 NeuronCores — banded/streamed KNN.

Host (numpy): edge detection, coordinate compaction, half-res EDT for
certified NN-distance upper bounds, per-tile pred *bands* (contiguous
index intervals guaranteed to contain all NN candidates both ways).
Bands are split to <=1024 cols, rank-matched across the 8 cores (sorted
by width; width at rank k = max over cores), and the rhs operand is
PRE-GATHERED per core into a position-packed schedule array, so the
device program has only compile-time offsets while every core computes
its own (tight) bands.

Device (raw Bass, SPMD over 8 cores, 2 pair-slots per core):
  PE : per job, matmuls of 6-row augmented operands over its W_k band
       -> PSUM = -(d^2)/4 exactly (two jobs per PSUM bank-group)
  ACT: one activation Copy (scale 2^-12) per PSUM group -> fp16 ring
  DVE: two batched fold ops per 4-job group (gth->pred NN partials)
  DMA: fp16 blocks stream to DRAM per group (pred->gth NN finished as a
       128-way column max on host), dg partials stream via GPSIMD queue
Host: column maxes, scatter-max into pred space, sqrt, means, nanmean.

Pads use a far sentinel coordinate so they always lose the max.
"""

import numpy as np

H = 256
W_IMG = 256
BC = 16
N_CORES = 8
SLOTS = 2
G_TILE = 128
QUANT = 128
W_CAP = 1024     # max job width (2 jobs <= 2048 fp32 = 4 PSUM banks)
FOLD_B = 4       # jobs per DVE fold group
NB = 4           # d2s ring depth (fold-group slots)
SENT = 16384.0
D2_SCALE = 2.0 ** -12
D2_BACK = -4.0 * 4096.0
EDT_SLACK = 1.5


def _edge_maps(x):
    m = x > 0.5
    p = np.pad(m, ((0, 0), (1, 1), (1, 1)), constant_values=True)
    e = np.ones_like(m)
    for dy in range(3):
        for dx in range(3):
            e &= p[:, dy:dy + H, dx:dx + W_IMG]
    return m & ~e


def _edt_half(mask):
    """Exact EDT of the half-res occupancy of `mask` ([256,256] bool)."""
    m = mask.reshape(128, 2, 128, 2).any(axis=(1, 3))
    BIG = np.float32(1e9)
    col = np.where(m, np.float32(0.0), BIG)
    ar = np.arange(128, dtype=np.float32)
    d2 = (ar[:, None] - ar[None, :]) ** 2
    D1 = (d2[:, :, None] + col[None, :, :]).min(1)
    D2 = (D1[:, None, :] + d2[None, :, :]).min(2)
    return np.sqrt(D2)


def _nn_upper_bound(edt_half_other, ys, xs):
    return 2.0 * edt_half_other[ys // 2, xs // 2] + EDT_SLACK


def _aug_g(cy, cx):
    n = cy.shape[0]
    out = np.zeros((6, n), np.float32)
    sq = cy * cy + cx * cx
    b1 = np.floor(sq / 256.0)
    b0 = sq - b1 * 256.0
    out[0] = cy * 0.5
    out[1] = cx * 0.5
    out[2] = -b1
    out[3] = -b0
    out[4] = -64.0
    out[5] = -0.25
    return out


def _aug_p(cy, cx):
    n = cy.shape[0]
    out = np.zeros((6, n), np.float32)
    sq = cy * cy + cx * cx
    b1 = np.floor(sq / 256.0)
    b0 = sq - b1 * 256.0
    out[0] = cy
    out[1] = cx
    out[2] = 64.0
    out[3] = 0.25
    out[4] = b1
    out[5] = b0
    return out


def _pair_bands(gy, gx, py, px, u_g, v_p, T):
    n_g, n_p = len(gy), len(py)
    bands = []
    for t in range(T):
        a, b = (t * n_g) // T, ((t + 1) * n_g) // T
        if b <= a:
            bands.append((0, 1))
            continue
        ymin, ymax = gy[a:b].min(), gy[a:b].max()
        U = u_g[a:b].max()
        lo1 = np.searchsorted(py, ymin - U, 'left')
        hi1 = np.searchsorted(py, ymax + U, 'right')
        sel = (py + v_p >= ymin) & (py - v_p <= ymax)
        nz = np.nonzero(sel)[0]
        if len(nz):
            lo2, hi2 = nz[0], nz[-1] + 1
        else:
            lo2, hi2 = lo1, hi1
        lo, hi = int(min(lo1, lo2)), int(max(hi1, hi2))
        hi = max(hi, lo + 1)
        bands.append((lo, hi))
    return bands


def _pair_jobs(bands):
    """Split bands into jobs (tile, lo, w<=W_CAP), sorted by width desc."""
    jobs = []
    for t, (lo, hi) in enumerate(bands):
        wid = hi - lo
        n_sp = -(-wid // W_CAP)
        base = -(-(-(-wid // n_sp)) // QUANT) * QUANT
        off = lo
        while off < hi:
            w = min(base, (-(-(hi - off) // QUANT)) * QUANT)
            jobs.append((t, off, w))
            off += w
    jobs.sort(key=lambda j: -j[2])
    return jobs


def _plan_slot(jobs_8):
    """jobs_8: jobs list per pair of the slot. Returns (widths, offsets)."""
    nrank = max(len(j) for j in jobs_8)
    widths = []
    for k in range(nrank):
        widths.append(max((j[k][2] for j in jobs_8 if len(j) > k),
                          default=QUANT))
    # fold-group padding: widths desc -> pad to group max
    for g0 in range(0, nrank, FOLD_B):
        wm = widths[g0]
        for k in range(g0, min(g0 + FOLD_B, nrank)):
            widths[k] = wm
    offs = np.concatenate([[0], np.cumsum(widths)]).astype(int)
    return widths, offs


def _build_program(slot_w, slot_T):
    """slot_w: per slot, list of common rank widths.  slot_T: gaug tiles
    per slot (incl sentinel tile)."""
    from contextlib import ExitStack
    import concourse.bass as bass
    import concourse.mybir as mybir

    f32 = mybir.dt.float32
    f16 = mybir.dt.float16
    bf16 = mybir.dt.bfloat16

    nc = bass.Bass()
    wmax = max(max(w) for w in slot_w)
    C = [int(sum(w)) for w in slot_w]          # schedule cols per slot
    Cq = [c // 4 for c in C]                   # dg partial cols

    gaug_d, paug_d, dg_d, dp_d = [], [], [], []
    for s in range(SLOTS):
        gaug_d.append(nc.declare_dram_parameter(
            f"gaug{s}", [6, slot_T[s] * G_TILE], bf16, isOutput=False))
        paug_d.append(nc.declare_dram_parameter(
            f"paug{s}", [6, C[s]], bf16, isOutput=False))
        dg_d.append(nc.declare_dram_parameter(
            f"dg{s}", [G_TILE, Cq[s]], f16, isOutput=True))
        dp_d.append(nc.declare_dram_parameter(
            f"dp{s}", [G_TILE, C[s]], f16, isOutput=True))

    # emission bookkeeping --------------------------------------------------
    # jobs in rank order per slot; psum groups = consecutive pairs;
    # fold groups = FOLD_B consecutive ranks (2 psum groups).
    pg_list = []   # (slot, ranks)
    fg_list = []   # (slot, ranks, pg ids, fg width)
    for s, ws in enumerate(slot_w):
        k = 0
        base_pg = len(pg_list)
        while k < len(ws):
            pg_list.append((s, list(range(k, min(k + 2, len(ws))))))
            k += 2
        k = 0
        pgi = base_pg
        while k < len(ws):
            take = min(FOLD_B, len(ws) - k)
            npg = (take + 1) // 2
            fg_list.append((s, list(range(k, k + take)),
                            list(range(pgi, pgi + npg))))
            pgi += npg
            k += take
    n_fg = len(fg_list)
    fg_end = [2 * (i + 1) for i in range(n_fg)]   # dve ops per fg = 2
    acts_thru = []
    tot = 0
    for (s, ranks, pgs) in fg_list:
        tot += len(pgs)
        acts_thru.append(tot)
    slot_last_fg = {}
    for i, (s, ranks, pgs) in enumerate(fg_list):
        slot_last_fg[s] = i
    # rank offsets per slot
    offs = [np.concatenate([[0], np.cumsum(w)]).astype(int) for w in slot_w]

    with ExitStack() as ctx:
        gs, ps, dg_st = [], [], []
        for s in range(SLOTS):
            gs.append(ctx.enter_context(
                nc.sbuf_tensor(f"gs{s}", [6, slot_T[s] * G_TILE], bf16)))
            ps.append(ctx.enter_context(
                nc.sbuf_tensor(f"ps{s}", [6, C[s]], bf16)))
            dg_st.append(ctx.enter_context(
                nc.sbuf_tensor(f"dgst{s}", [G_TILE, Cq[s]], f16)))
        pt = [ctx.enter_context(nc.psum_tensor(f"pt{i}", [G_TILE, 2048], f32))
              for i in range(2)]
        d2s = ctx.enter_context(
            nc.sbuf_tensor("d2s", [G_TILE, NB, FOLD_B, wmax], f16))
        fd1 = ctx.enter_context(
            nc.sbuf_tensor("fd1", [G_TILE, 2, FOLD_B, wmax // 2], f16))

        dma_sems = [ctx.enter_context(nc.semaphore(f"dma_in{s}"))
                    for s in range(SLOTS)]
        pe_sem = ctx.enter_context(nc.semaphore("pe_done"))
        act_sem = ctx.enter_context(nc.semaphore("act_done"))
        dve_sem = ctx.enter_context(nc.semaphore("dve_done"))
        out_sem = ctx.enter_context(nc.semaphore("dma_out"))
        dgo_sem = ctx.enter_context(nc.semaphore("dma_dg_out"))
        block = ctx.enter_context(nc.Block())

        # map tile index per (slot, rank): provided by caller via closure
        # (gaug layout); the tile for rank k is encoded in gaug directly --
        # the device just uses lhsT slice per rank from a lookup list.
        # We pass it through slot_w's companion structure set below.
        rank_tile = _build_program.rank_tile  # [slot][rank] -> gaug tile idx

        @block.sync
        def _(sync):
            for s in range(SLOTS):
                sync.dma_start(gs[s][:], gaug_d[s][:]).then_inc(dma_sems[s], 16)
                sync.dma_start(ps[s][:], paug_d[s][:]).then_inc(dma_sems[s], 16)
            # dp stream per fold group
            for i, (s, ranks, pgs) in enumerate(fg_list):
                sync.wait_ge(act_sem, acts_thru[i])
                o0, o1 = offs[s][ranks[0]], offs[s][ranks[-1] + 1]
                W_g = slot_w[s][ranks[0]]
                src = d2s[:, i % NB, 0:len(ranks), 0:W_g]
                sync.dma_start(dp_d[s][:, o0:o1], src).then_inc(out_sem, 16)

        @block.gpsimd
        def _(gpsimd):
            # dg partial stream per fold group (separate queue from sync)
            for i, (s, ranks, pgs) in enumerate(fg_list):
                gpsimd.wait_ge(dve_sem, fg_end[i])
                o0, o1 = offs[s][ranks[0]] // 4, offs[s][ranks[-1] + 1] // 4
                gpsimd.dma_start(
                    dg_d[s][:, o0:o1], dg_st[s][:, o0:o1],
                ).then_inc(dgo_sem, 16)

        @block.tensor
        def _(tensor):
            cur_slot = -1
            for pg, (s, ranks) in enumerate(pg_list):
                if s != cur_slot:
                    tensor.wait_ge(dma_sems[s], 32)
                    cur_slot = s
                if pg >= 2:
                    tensor.wait_ge(act_sem, pg - 1)
                mm = None
                o = 0
                for k in ranks:
                    Wk = slot_w[s][k]
                    t = rank_tile[s][k]
                    lhsT = gs[s][:, t * G_TILE:(t + 1) * G_TILE]
                    done = 0
                    while done < Wk:
                        room = 512 - ((o + done) % 512)
                        w = min(room, Wk - done)
                        mm = nc.tensor.matmul(
                            pt[pg % 2][:, o + done:o + done + w],
                            lhsT,
                            ps[s][:, offs[s][k] + done:offs[s][k] + done + w],
                            start=True, stop=True,
                        )
                        done += w
                    o += Wk
                mm.then_inc(pe_sem, 1)

        @block.scalar
        def _(scalar):
            fg_of_pg = {}
            for fgi, (s, ranks, pgs) in enumerate(fg_list):
                for p in pgs:
                    fg_of_pg[p] = (fgi, pgs)
            for pg, (s, ranks) in enumerate(pg_list):
                scalar.wait_ge(pe_sem, pg + 1)
                fgi, pgs = fg_of_pg[pg]
                if fgi >= NB and pgs.index(pg) == 0:
                    # ring slot reuse: folds + dp stream of fg-NB done
                    scalar.wait_ge(dve_sem, fg_end[fgi - NB])
                    scalar.wait_ge(out_sem, 16 * (fgi - NB + 1))
                W_g = slot_w[s][ranks[0]]
                half = pgs.index(pg)
                nw = len(ranks) * W_g
                dst = d2s[:, fgi % NB, half * 2:half * 2 + len(ranks), :W_g]
                src = pt[pg % 2][:, 0:nw].rearrange(
                    "p (a b) -> p a b", a=len(ranks))
                nc.scalar.activation(
                    dst, src,
                    mybir.ActivationFunctionType.Copy, scale=D2_SCALE,
                ).then_inc(act_sem, 1)

        @block.vector
        def _(vector):
            for fgi, (s, ranks, pgs) in enumerate(fg_list):
                vector.wait_ge(act_sem, acts_thru[fgi])
                W_g = slot_w[s][ranks[0]]
                nt = len(ranks)
                r = fgi % NB
                h1, h2 = W_g // 2, W_g // 4
                nc.vector.tensor_max(
                    fd1[:, fgi % 2, 0:nt, 0:h1],
                    d2s[:, r, 0:nt, 0:h1],
                    d2s[:, r, 0:nt, h1:W_g],
                ).then_inc(dve_sem, 1)
                o0 = offs[s][ranks[0]] // 4
                dst = dg_st[s][:, o0:o0 + nt * h2].rearrange(
                    "p (a b) -> p a b", a=nt)
                nc.vector.tensor_max(
                    dst,
                    fd1[:, fgi % 2, 0:nt, 0:h2],
                    fd1[:, fgi % 2, 0:nt, h2:h1],
                ).then_inc(dve_sem, 1)

    return nc


def _loss_from_nn(d_g, d_p, n_g, n_p):
    with np.errstate(divide="ignore", invalid="ignore", over="ignore"):
        gth2pred = d_g.sum() / n_g if n_g > 0 else np.float64(np.nan)
        pred2gth = d_p.sum() / n_p if n_p > 0 else np.float64(np.nan)
        ahd = (gth2pred + pred2gth) / 2.0
        if n_g == 0 and n_p == 0:
            ahd = np.float64(np.nan)
        return 1.0 - 1.0 / (1.0 + ahd)


RUN_OPTS = {}
LAST_RES = None
LAST_INFO = {}


def kernel(gth, pred):
    from concourse.bass_utils import run_bass_kernel_spmd
    import ml_dtypes

    gth = np.asarray(gth, np.float32).reshape(BC, H, W_IMG)
    pred = np.asarray(pred, np.float32).reshape(BC, H, W_IMG)

    gedge = _edge_maps(gth)
    pedge = _edge_maps(pred)

    pts, pair_bands = [], []
    for i in range(BC):
        gy, gx = np.nonzero(gedge[i])
        py, px = np.nonzero(pedge[i])
        pts.append((gy.astype(np.int64), gx.astype(np.int64),
                    py.astype(np.int64), px.astype(np.int64)))
        n_g, n_p = len(gy), len(py)
        if n_g and n_p:
            u_g = _nn_upper_bound(_edt_half(pedge[i]), gy, gx)
            v_p = _nn_upper_bound(_edt_half(gedge[i]), py, px)
            T0 = -(-n_g // G_TILE)
            pair_bands.append(None)  # placeholder, fill after T known
        else:
            pair_bands.append('empty')

    n_gs = [len(p[0]) for p in pts]
    T = max(1, -(-max(n_gs) // G_TILE))
    for i in range(BC):
        gy, gx, py, px = pts[i]
        if pair_bands[i] == 'empty':
            pair_bands[i] = [(0, max(1, len(py)))] * T
        else:
            u_g = _nn_upper_bound(_edt_half(pedge[i]), gy, gx)
            v_p = _nn_upper_bound(_edt_half(gedge[i]), py, px)
            pair_bands[i] = _pair_bands(gy, gx, py, px, u_g, v_p, T)

    pair_jobs = [_pair_jobs(b) for b in pair_bands]
    cost = [sum(j[2] for j in jb) for jb in pair_jobs]
    order = sorted(range(BC), key=lambda i: -cost[i])
    slot_pairs = [order[0::2], order[1::2]]
    assign = [[slot_pairs[0][c], slot_pairs[1][N_CORES - 1 - c]]
              for c in range(N_CORES)]

    slot_w, slot_offs = [], []
    for s in range(SLOTS):
        w, o = _plan_slot([pair_jobs[i] for i in slot_pairs[s]])
        slot_w.append(w)
        slot_offs.append(o)

    # gaug tile layout: T quantile tiles + 1 sentinel tile per slot
    slot_T = [T + 1, T + 1]
    rank_tile = []
    for s in range(SLOTS):
        # rank k uses the tile of whichever pair; tile index must be common
        # across cores -> store per-rank tile as the job's tile for EACH core
        # in ITS OWN gaug. But lhsT slice index must be compile-time common!
        # Solution: gaug layout per core is REORDERED so that rank k's tile
        # data sits at gaug position k. ranks can exceed T (splits reuse the
        # same tile for several ranks; sentinel ranks use sentinel data).
        rank_tile.append(list(range(len(slot_w[s]))))
    slot_T = [len(slot_w[s]) for s in range(SLOTS)]
    _build_program.rank_tile = rank_tile

    nc = _build_program(slot_w, slot_T)

    in_maps = []
    core_maps = []   # per core, per slot: list per rank of (pair, tile, lo, nreal)
    for c in range(N_CORES):
        m = {}
        cmaps = []
        for s in range(SLOTS):
            i = assign[c][s]
            gy, gx, py, px = pts[i]
            n_g, n_p = len(gy), len(py)
            jobs = pair_jobs[i]
            nrank = len(slot_w[s])
            C_s = int(slot_offs[s][-1])
            # gaug: rank-ordered tiles (sentinel pad rows inside tiles)
            cyg = np.full(nrank * G_TILE, SENT, np.float32)
            cxg = np.full(nrank * G_TILE, SENT, np.float32)
            rmap = []
            for k in range(nrank):
                if k < len(jobs):
                    t, lo, wreal = jobs[k]
                    a, b = (t * n_g) // T, ((t + 1) * n_g) // T
                    cyg[k * G_TILE:k * G_TILE + (b - a)] = gy[a:b] - 128.0
                    cxg[k * G_TILE:k * G_TILE + (b - a)] = gx[a:b] - 128.0
                    rmap.append((t, lo, a, b))
                else:
                    rmap.append(None)
            # paug: gathered band columns per rank
            cyp = np.full(C_s, SENT, np.float32)
            cxp = np.full(C_s, SENT, np.float32)
            for k in range(nrank):
                if rmap[k] is None:
                    continue
                t, lo, a, b = rmap[k]
                Wk = slot_w[s][k]
                nreal = max(0, min(Wk, n_p - lo))
                o = int(slot_offs[s][k])
                cyp[o:o + nreal] = py[lo:lo + nreal] - 128.0
                cxp[o:o + nreal] = px[lo:lo + nreal] - 128.0
                rmap[k] = (t, lo, a, b, nreal)
            m[f"gaug{s}"] = _aug_g(cyg, cxg).astype(ml_dtypes.bfloat16)
            m[f"paug{s}"] = _aug_p(cyp, cxp).astype(ml_dtypes.bfloat16)
            cmaps.append(rmap)
        in_maps.append(m)
        core_maps.append(cmaps)

    res = run_bass_kernel_spmd(nc, in_maps, list(range(N_CORES)), **RUN_OPTS)
    global LAST_RES, LAST_INFO
    LAST_RES = res
    LAST_INFO = {"slot_w": slot_w, "assign": assign, "T": T}
    results = res.results

    losses = np.full(BC, np.nan, np.float64)
    for c in range(N_CORES):
        for s in range(SLOTS):
            i = assign[c][s]
            gy, gx, py, px = pts[i]
            n_g, n_p = len(gy), len(py)
            if n_g == 0 and n_p == 0:
                continue
            rmap = core_maps[c][s]
            dg_raw = np.asarray(results[c][f"dg{s}"], np.float32)
            dp_raw = np.asarray(results[c][f"dp{s}"], np.float32)
            colmax = dp_raw.max(axis=0)
            val_g = np.full((T, G_TILE), -np.inf, np.float32)
            dpv = np.full(max(n_p, 1), -np.inf, np.float32)
            for k in range(len(slot_w[s])):
                if rmap[k] is None:
                    continue
                t, lo, a, b, nreal = rmap[k]
                Wk = slot_w[s][k]
                o = int(slot_offs[s][k])
                blk = dg_raw[:, o // 4:(o + Wk) // 4].max(axis=1)
                val_g[t] = np.maximum(val_g[t], blk)
                if nreal > 0:
                    dpv[lo:lo + nreal] = np.maximum(
                        dpv[lo:lo + nreal], colmax[o:o + nreal])
            dgv = np.empty(max(n_g, 1), np.float32)
            for t in range(T):
                a, b = (t * n_g) // T, ((t + 1) * n_g) // T
                dgv[a:b] = val_g[t, :b - a]
            d_g = np.sqrt(np.maximum(D2_BACK * dgv[:n_g].astype(np.float64), 0.0))
            d_p = np.sqrt(np.maximum(D2_BACK * dpv[:n_p].astype(np.float64), 0.0))
            losses[i] = _loss_from_nn(d_g, d_p, n_g, n_p)

    return np.float32(np.nanmean(losses.astype(np.float32)))
